# revision 19
# baseline (speedup 1.0000x reference)
"""Trainium2 Bass kernel for nn_RadialBasisArbitraryLayer.

phi[b,c,h,w] = sum_n wendland(|pix(h,w) - p[b,n]| / c_b) * alpha[b,n,c]
  wendland(d) = (1-d)^4 (4d+1) for d<1 else 0;  c_b = 2 * maxmin-NN-dist.

Strategy (8 NeuronCores, batch b -> core b):
- dist^2 via a K=12 fp16 split-feature matmul on TensorE:
  dist^2 = (x^2+y^2) + (px^2+py^2) - 2 x px - 2 y py, with the point-side
  values 3-way fp16 split (pixel coords are integers -> exact fp16) so the
  PE's fp32 accumulation reconstructs dist^2 to ~1e-3 absolute.
- compact support: c_b <= ~30 px, so each 32-row band of the image sees
  <= 64 control points. Host gathers per-band active points; two bands'
  point sets stack into one 128-partition tile (2 pixels per column),
  halving every per-column engine cost.
- elementwise (ScalarE sqrt + VectorE/GpSimdE fp16 chain):
  d = sqrt(dist^2)/c; t = relu(1-d) via tau = min(d,1)-1 (t^2 = tau^2);
  w = 5 t^4 - 4 t^5 with t^5 = -(t^4 * tau).
- contraction over points on TensorE: psum += (5 alpha)^T t4 + (-4 alpha)^T t5
  with block-diagonal weights producing both stacked bands' outputs.
"""
import sys
import os

for _p in ('/opt/trn_rl_repo',):
    if os.path.isdir(_p) and _p not in sys.path:
        sys.path.insert(0, _p)

import numpy as np

# problem constants (hardcoded per spec)
B = 8
N = 160
CH = 2
H = 256
W = 256
C_FACTOR = 2.0
BIG = 1e10

R_BAND = 4           # image rows per band
NBANDS = H // R_BAND  # 64
EPS = 0.1            # dist^2 epsilon (keeps sqrt positive vs split residual)
PAD_XY = -255.0      # pad-point coords: >=255px from any pixel -> w = 0
FG = 1024            # columns per processing group (= one band)

_CACHE = {}
LAST_EXEC_TIME_NS = None


def _maybe_install_trace_hook():
    """Best-effort NTFF profile hook for RBF_TRACE=1 runs (dev only)."""
    try:
        import types
        import antenv
        import trn_agent_boot.trn_boot as tb

        hook = tb._ntff_profile_via_ctypes('/opt/axon/libaxon_pjrt.so')
        mod = types.ModuleType('antenv.axon_hooks')
        mod._hook = hook
        mod.set_axon_ntff_profile_hook = lambda h: setattr(mod, '_hook', h)
        mod.get_axon_ntff_profile_hook = lambda: mod._hook
        sys.modules['antenv.axon_hooks'] = mod
        antenv.axon_hooks = mod
        import concourse.bass_utils as bu
        bu.upload_artifacts = lambda tmpdir: f"file://{tmpdir}"
        return True
    except Exception:
        return False


# ---------------------------------------------------------------- patches --
def _apply_patches():
    """Walrus in this container accepts at most ONE sync-wait per
    instruction; Tile assumes more. Split excess waits onto same-engine
    Drain (ctrl-nop) instructions, and emit the tail drain one-wait-per-
    drain."""
    import bass_rust
    import concourse.bass as bass
    import concourse.tile as tile
    from concourse.vector_clock import ScopedClock

    if getattr(tile.TileContext, "_rbf_patched", False):
        return

    def _drain_and_barrier(self, tick_clock, wait_clock):
        nc = self.nc
        drain_inst = nc.sync.drain()
        wait_clock.add_sem_waits(
            drain_inst.ins, ScopedClock({None: tick_clock.global_clock})
        )
        si = drain_inst.ins.sync_info
        waits = list(si.on_wait) if si is not None else []
        if len(waits) > 1:
            drain_inst.ins.sync_info.on_wait = waits[:1]
            for w in waits[1:]:
                extra = nc.sync.drain()
                extra.ins.sync_info = bass_rust.SyncInfo(on_wait=[w], on_update=[])
        nc.all_engine_barrier()
        assert self.sems is not None
        popped = nc._tile_sem_poison_stack.pop()
        assert popped is self._sem_poison
        nc.clear_and_free_semaphores(list(self.sems.allocated().values()))
        nc.all_engine_barrier()

    tile.TileContext._drain_and_barrier = _drain_and_barrier

    def _split_waits_json(raw: bytes) -> bytes:
        import orjson

        mod = orjson.loads(raw)
        ctr = 0
        for fn in mod.get("functions", []):
            for blk in fn.get("blocks", []):
                insts = blk.get("instructions", [])
                out = []
                changed = False
                for inst in insts:
                    si = inst.get("sync_info")
                    waits = si.get("on_wait", []) if si else []
                    if len(waits) > 1:
                        changed = True
                        si["on_wait"] = waits[-1:]
                        for w in waits[:-1]:
                            ctr += 1
                            out.append({
                                "name": f"WSPLIT-{ctr}",
                                "opcode": "Drain",
                                "engine": inst["engine"],
                                "ins": [],
                                "outs": [],
                                "sync_info": {"on_wait": [w], "on_update": []},
                            })
                    out.append(inst)
                if changed:
                    blk["instructions"] = out
        return orjson.dumps(mod)

    _orig = bass.Bass.to_json_bytes

    def to_json_bytes(self):
        return _split_waits_json(_orig(self))

    bass.Bass.to_json_bytes = to_json_bytes
    tile.TileContext._rbf_patched = True


# ------------------------------------------------------------ host helpers --
def _support_radius(cpoint_loc):
    diff = cpoint_loc[:, :, None, :] - cpoint_loc[:, None, :, :]
    sq = (diff * diff).sum(-1)
    sq = sq + np.eye(N, dtype=sq.dtype) * BIG
    return np.sqrt(sq.min(2).max(1)) * C_FACTOR  # [B]


def _split3(v):
    h = v.astype(np.float16)
    r = v - h.astype(np.float32)
    m = r.astype(np.float16)
    l = (r - m.astype(np.float32)).astype(np.float16)
    return h, m, l


def _point_features(px, py):
    """[12, K] fp16: [4,4,4, s4h,s4m,s4l, -2pxh,-2pxm,-2pxl, -2pyh,-2pym,-2pyl]"""
    sp4 = (px * px + py * py + EPS) * np.float32(0.25)
    s4h, s4m, s4l = _split3(sp4)
    four = np.full_like(px, 4.0, dtype=np.float16)

    def m2(v):
        return (np.float32(-2.0) * v.astype(np.float32)).astype(np.float16)

    pxh, pxm, pxl = _split3(px)
    pyh, pym, pyl = _split3(py)
    return np.stack([four, four, four, s4h, s4m, s4l,
                     m2(pxh), m2(pxm), m2(pxl),
                     m2(pyh), m2(pym), m2(pyl)]).astype(np.float16)


def _pixel_features():
    """[NBANDS, 12, R_BAND*W] fp16 pixel features, row-major per band."""
    yy, xx = np.meshgrid(np.arange(H, dtype=np.float32),
                         np.arange(W, dtype=np.float32), indexing='ij')
    x = xx.reshape(-1)
    y = yy.reshape(-1)
    gp4 = (x * x + y * y) * np.float32(0.25)
    g4h, g4m, g4l = _split3(gp4)
    four = np.full_like(x, 4.0, dtype=np.float16)
    xb = x.astype(np.float16)
    yb = y.astype(np.float16)
    feats = np.stack([g4h, g4m, g4l, four, four, four,
                      xb, xb, xb, yb, yb, yb]).astype(np.float16)  # [12, H*W]
    return feats.reshape(12, NBANDS, R_BAND * W).transpose(1, 0, 2).copy()


def _gather_bands(cp, al, c, kpts):
    """Per band: active point features [12,kpts] fp16 and 5a/-4a [kpts,2] f32.
    Returns (ptf [NBANDS,12,kpts], a5 [NBANDS,kpts,2], a4 [NBANDS,kpts,2],
    max_active)."""
    px, py = cp[:, 0], cp[:, 1]
    ptf = np.zeros((NBANDS, 12, kpts), np.float16)
    a5 = np.zeros((NBANDS, kpts, 2), np.float32)
    a4 = np.zeros((NBANDS, kpts, 2), np.float32)
    max_active = 0
    for i in range(NBANDS):
        y0, y1 = i * R_BAND, i * R_BAND + R_BAND - 1
        dy = np.maximum(np.maximum(y0 - py, py - y1), 0.0)
        idx = np.nonzero(dy < c)[0]
        max_active = max(max_active, len(idx))
        if len(idx) > kpts:
            return None, None, None, max_active
        gx = np.full(kpts, PAD_XY, np.float32)
        gy = np.full(kpts, PAD_XY, np.float32)
        gx[:len(idx)] = px[idx]
        gy[:len(idx)] = py[idx]
        ptf[i] = _point_features(gx, gy)
        a5[i, :len(idx)] = al[idx]
        a4[i, :len(idx)] = al[idx]
    return ptf, a5, a4, max_active


# ------------------------------------------------------------ build module --
def _build_module_flat(kpts):
    """One 4-row band (1024 px) per column group; kpts<=128 gathered points
    on partitions. DMAs batched: all weights preloaded, pixel features in
    8-band chunks, outputs staged in SBUF and written per 16 bands.
    Contraction accumulates 4 bands into one [8, FG] psum via zero-padded
    block lhsT; ScalarE: sqrt + relu; VectorE: q/t4/t5; GpSimd: psum->stage
    copies."""
    import concourse.bass as bass
    import concourse.tile as tile
    import concourse.mybir as mybir

    NGRP = NBANDS // 4           # 16 groups of 4 bands
    NSTG = 4                     # groups per output stage buffer

    nc = bass.Bass()
    f16, f32 = mybir.dt.float16, mybir.dt.float32
    pxf_d = nc.dram_tensor("pxf", [NBANDS, 12, FG], f16, kind="ExternalInput")
    ptf_d = nc.dram_tensor("ptf", [NBANDS, 12, kpts], f16, kind="ExternalInput")
    al5_d = nc.dram_tensor("al5", [NBANDS, kpts, 8], f16, kind="ExternalInput")
    al4_d = nc.dram_tensor("al4", [NBANDS, kpts, 8], f16, kind="ExternalInput")
    sc_d = nc.dram_tensor("sc", [128, 1], f32, kind="ExternalInput")
    out_d = nc.dram_tensor("out", [CH, H * W], f32, kind="ExternalOutput")

    with tile.TileContext(nc) as tc:
        with (
            tc.tile_pool(name="weights", bufs=1) as wpool,
            tc.tile_pool(name="io", bufs=2) as iopool,
            tc.tile_pool(name="chain", bufs=3) as chpool,
            tc.tile_pool(name="stage", bufs=2) as stpool,
            tc.tile_pool(name="psum", bufs=2, space=bass.MemorySpace.PSUM) as pspool,
            tc.tile_pool(name="psumd2", bufs=1, space=bass.MemorySpace.PSUM) as pd2pool,
        ):
            s_sc = wpool.tile([128, 1], f32, tag="sc")
            nc.sync.dma_start(s_sc[:], sc_d[:])
            s_ptf = wpool.tile([12, NBANDS * kpts], f16, tag="ptf")
            nc.sync.dma_start(
                s_ptf.rearrange("f (b k) -> f b k", b=NBANDS),
                ptf_d.rearrange("b f k -> f b k"))
            s_al5 = wpool.tile([kpts, NBANDS * 8], f16, tag="al5")
            nc.sync.dma_start(
                s_al5.rearrange("k (b c) -> k b c", b=NBANDS),
                al5_d.rearrange("b k c -> k b c"))
            s_al4 = wpool.tile([kpts, NBANDS * 8], f16, tag="al4")
            nc.sync.dma_start(
                s_al4.rearrange("k (b c) -> k b c", b=NBANDS),
                al4_d.rearrange("b k c -> k b c"))

            for G in range(NGRP):
                if G % 2 == 0:
                    # pixel features for the next 8 bands in one DMA
                    s_pxf = iopool.tile([12, 8 * FG], f16, tag="pxf")
                    nc.sync.dma_start(
                        s_pxf.rearrange("f (b p) -> f b p", b=8),
                        pxf_d[8 * (G // 2):8 * (G // 2) + 8].rearrange(
                            "b f p -> f b p"))
                if G % NSTG == 0:
                    stage = stpool.tile([8, NSTG * FG], f32, tag="stage")

                ps_out = pspool.tile([8, FG], f32, tag="out")
                for half in range(2):                 # two band-pairs per group
                    i0 = 2 * half
                    bandP = [4 * G + i0, 4 * G + i0 + 1]
                    # wide chain tiles covering both bands (2*FG columns)
                    d_t = chpool.tile([kpts, 2 * FG], f16, tag="d")
                    t_t = chpool.tile([kpts, 2 * FG], f16, tag="t")
                    q = chpool.tile([kpts, 2 * FG], f16, tag="q")
                    t4 = chpool.tile([kpts, 2 * FG], f16, tag="t4")
                    t5 = chpool.tile([kpts, 2 * FG], f16, tag="t5")
                    ps_d2 = pd2pool.tile([kpts, 2 * FG], f32, tag="d2")
                    for k, band in enumerate(bandP):
                        ptcol = slice(band * kpts, band * kpts + kpts)
                        for j in range(FG // 512):
                            sl = slice(k * FG + j * 512, k * FG + (j + 1) * 512)
                            pxsl = slice((band % 8) * FG + j * 512,
                                         (band % 8) * FG + (j + 1) * 512)
                            nc.tensor.matmul(ps_d2[:, sl], s_ptf[:, ptcol],
                                             s_pxf[:, pxsl], start=True,
                                             stop=True)
                    nc.scalar.activation(
                        d_t[:], ps_d2[:],
                        mybir.ActivationFunctionType.Sqrt,
                        bias=0.0, scale=s_sc[0:kpts, :])
                    nc.scalar.activation(t_t[:], d_t[:],
                                         mybir.ActivationFunctionType.Relu,
                                         bias=1.0, scale=-1.0)
                    nc.vector.tensor_tensor(q[:], t_t[:], t_t[:],
                                            mybir.AluOpType.mult)
                    nc.vector.tensor_tensor(t4[:], q[:], q[:],
                                            mybir.AluOpType.mult)
                    nc.vector.tensor_tensor(t5[:], t4[:], t_t[:],
                                            mybir.AluOpType.mult)
                    for k, band in enumerate(bandP):
                        i = i0 + k
                        alcol = slice(band * 8, band * 8 + 8)
                        for j in range(FG // 512):
                            sl = slice(j * 512, (j + 1) * 512)
                            wsl = slice(k * FG + j * 512, k * FG + (j + 1) * 512)
                            nc.tensor.matmul(ps_out[:, sl], s_al5[:, alcol],
                                             t4[:, wsl], start=(i == 0),
                                             stop=False)
                        for j in range(FG // 512):
                            sl = slice(j * 512, (j + 1) * 512)
                            wsl = slice(k * FG + j * 512, k * FG + (j + 1) * 512)
                            nc.tensor.matmul(ps_out[:, sl], s_al4[:, alcol],
                                             t5[:, wsl], start=False,
                                             stop=(i == 3))

                gg = G % NSTG
                nc.vector.tensor_copy(stage[:, gg * FG:(gg + 1) * FG], ps_out[:])
                if gg == NSTG - 1:
                    # stage[2i+ch, gg*FG+p] -> out[ch, base + gg*4FG + i*FG + p]
                    base = (G - NSTG + 1) * 4 * FG
                    st_r = stage.rearrange("(i r) (g p) -> r i g p", r=2, g=NSTG)
                    for ch in range(CH):
                        dst = out_d[ch, base:base + 4 * NSTG * FG].rearrange(
                            "(g i p) -> i g p", g=NSTG, i=4, p=FG)
                        nc.sync.dma_start(dst, st_r[ch])
    return nc


# ------------------------------------------------------------------ kernel --
def kernel(cpoint_loc: np.ndarray, alpha: np.ndarray) -> np.ndarray:
    assert cpoint_loc.shape == (B, N, 2) and alpha.shape == (B, N, CH)
    cp = np.asarray(cpoint_loc, np.float32)
    al = np.asarray(alpha, np.float32)

    _apply_patches()
    from concourse.bass_utils import run_bass_kernel_spmd

    c = _support_radius(cp)                      # [B]
    pxf = _pixel_features()                      # shared [NBANDS, 12, FG]

    # probe active counts, pick a uniform kpts across cores
    mx_all = 0
    for b in range(B):
        _, _, _, mx = _gather_bands(cp[b], al[b], c[b], 160)
        mx_all = max(mx_all, mx)
    if mx_all > 128:
        raise NotImplementedError(
            f"band active count {mx_all} > 128 unsupported")
    kpts = max(32, int(np.ceil(mx_all / 32.0)) * 32)
    gathered = [_gather_bands(cp[b], al[b], c[b], kpts) for b in range(B)]

    key = ("flat", kpts)
    if key not in _CACHE:
        _CACHE[key] = _build_module_flat(kpts)
    nc = _CACHE[key]

    in_maps = []
    for b in range(B):
        ptf, a5, a4, _ = gathered[b]
        sc = np.full((128, 1), 1.0 / (c[b] * c[b]), np.float32)
        # block layout: band i of its 4-band group owns lhsT cols 2i..2i+1
        al5 = np.zeros((NBANDS, kpts, 8), np.float16)
        al4 = np.zeros((NBANDS, kpts, 8), np.float16)
        for band in range(NBANDS):
            i = band % 4
            al5[band, :, 2 * i:2 * i + 2] = (5.0 * a5[band]).astype(np.float16)
            al4[band, :, 2 * i:2 * i + 2] = (-4.0 * a4[band]).astype(np.float16)
        in_maps.append({"pxf": pxf, "ptf": ptf, "al5": al5, "al4": al4,
                        "sc": sc})

    trace = bool(os.environ.get("RBF_TRACE")) and _maybe_install_trace_hook()
    res = run_bass_kernel_spmd(nc, in_maps, core_ids=list(range(B)), trace=trace)
    global LAST_EXEC_TIME_NS
    if res.exec_time_ns is not None:
        LAST_EXEC_TIME_NS = res.exec_time_ns
    out = np.stack([r["out"] for r in res.results])     # [B, 2, H*W]
    return out.reshape(B, CH, H, W).astype(np.float32)


if __name__ == "__main__":
    rng = np.random.default_rng(0)
    cp = rng.random((B, N, 2), np.float32) * np.array([W - 1, H - 1], np.float32)
    a = rng.standard_normal((B, N, CH)).astype(np.float32)
    out = kernel(cp, a)
    print(out.shape, np.abs(out).max())


# revision 21
# speedup vs baseline: 1.2894x; 1.2894x over previous
"""Trainium2 Bass kernel for nn_RadialBasisArbitraryLayer.

phi[b,c,h,w] = sum_n wendland(|pix(h,w) - p[b,n]| / c_b) * alpha[b,n,c]
  wendland(d) = (1-d)^4 (4d+1) for d<1 else 0;  c_b = 2 * maxmin-NN-dist.

Strategy (8 NeuronCores, batch b -> core b):
- dist^2 via a K=12 fp16 split-feature matmul on TensorE:
  dist^2 = (x^2+y^2) + (px^2+py^2) - 2 x px - 2 y py, with the point-side
  values 3-way fp16 split (pixel coords are integers -> exact fp16) so the
  PE's fp32 accumulation reconstructs dist^2 to ~1e-3 absolute.
- compact support: c_b <= ~30 px, so each 32-row band of the image sees
  <= 64 control points. Host gathers per-band active points; two bands'
  point sets stack into one 128-partition tile (2 pixels per column),
  halving every per-column engine cost.
- elementwise (ScalarE sqrt + VectorE/GpSimdE fp16 chain):
  d = sqrt(dist^2)/c; t = relu(1-d) via tau = min(d,1)-1 (t^2 = tau^2);
  w = 5 t^4 - 4 t^5 with t^5 = -(t^4 * tau).
- contraction over points on TensorE: psum += (5 alpha)^T t4 + (-4 alpha)^T t5
  with block-diagonal weights producing both stacked bands' outputs.
"""
import sys
import os

for _p in ('/opt/trn_rl_repo',):
    if os.path.isdir(_p) and _p not in sys.path:
        sys.path.insert(0, _p)

import numpy as np

# problem constants (hardcoded per spec)
B = 8
N = 160
CH = 2
H = 256
W = 256
C_FACTOR = 2.0
BIG = 1e10

R_BAND = 4           # image rows per band
NBANDS = H // R_BAND  # 64
EPS = 0.1            # dist^2 epsilon (keeps sqrt positive vs split residual)
PAD_XY = -255.0      # pad-point coords: >=255px from any pixel -> w = 0
FG = 1024            # columns per processing group (= one band)

_CACHE = {}
LAST_EXEC_TIME_NS = None


def _maybe_install_trace_hook():
    """Best-effort NTFF profile hook for RBF_TRACE=1 runs (dev only)."""
    try:
        import types
        import antenv
        import trn_agent_boot.trn_boot as tb

        hook = tb._ntff_profile_via_ctypes('/opt/axon/libaxon_pjrt.so')
        mod = types.ModuleType('antenv.axon_hooks')
        mod._hook = hook
        mod.set_axon_ntff_profile_hook = lambda h: setattr(mod, '_hook', h)
        mod.get_axon_ntff_profile_hook = lambda: mod._hook
        sys.modules['antenv.axon_hooks'] = mod
        antenv.axon_hooks = mod
        import concourse.bass_utils as bu
        bu.upload_artifacts = lambda tmpdir: f"file://{tmpdir}"
        return True
    except Exception:
        return False


# ---------------------------------------------------------------- patches --
def _apply_patches():
    """Walrus in this container accepts at most ONE sync-wait per
    instruction; Tile assumes more. Split excess waits onto same-engine
    Drain (ctrl-nop) instructions, and emit the tail drain one-wait-per-
    drain."""
    import bass_rust
    import concourse.bass as bass
    import concourse.tile as tile
    from concourse.vector_clock import ScopedClock

    if getattr(tile.TileContext, "_rbf_patched", False):
        return

    def _drain_and_barrier(self, tick_clock, wait_clock):
        nc = self.nc
        drain_inst = nc.sync.drain()
        wait_clock.add_sem_waits(
            drain_inst.ins, ScopedClock({None: tick_clock.global_clock})
        )
        si = drain_inst.ins.sync_info
        waits = list(si.on_wait) if si is not None else []
        if len(waits) > 1:
            drain_inst.ins.sync_info.on_wait = waits[:1]
            for w in waits[1:]:
                extra = nc.sync.drain()
                extra.ins.sync_info = bass_rust.SyncInfo(on_wait=[w], on_update=[])
        nc.all_engine_barrier()
        assert self.sems is not None
        popped = nc._tile_sem_poison_stack.pop()
        assert popped is self._sem_poison
        nc.clear_and_free_semaphores(list(self.sems.allocated().values()))
        nc.all_engine_barrier()

    tile.TileContext._drain_and_barrier = _drain_and_barrier

    def _split_waits_json(raw: bytes) -> bytes:
        import orjson

        mod = orjson.loads(raw)
        ctr = 0
        for fn in mod.get("functions", []):
            for blk in fn.get("blocks", []):
                insts = blk.get("instructions", [])
                out = []
                changed = False
                for inst in insts:
                    si = inst.get("sync_info")
                    waits = si.get("on_wait", []) if si else []
                    if len(waits) > 1:
                        changed = True
                        si["on_wait"] = waits[-1:]
                        for w in waits[:-1]:
                            ctr += 1
                            out.append({
                                "name": f"WSPLIT-{ctr}",
                                "opcode": "EventSemaphore",
                                "engine": inst["engine"],
                                "ins": [],
                                "outs": [],
                                "sync_info": {"on_wait": [w], "on_update": []},
                            })
                    out.append(inst)
                if changed:
                    blk["instructions"] = out
        return orjson.dumps(mod)

    _orig = bass.Bass.to_json_bytes

    def to_json_bytes(self):
        return _split_waits_json(_orig(self))

    bass.Bass.to_json_bytes = to_json_bytes
    tile.TileContext._rbf_patched = True


# ------------------------------------------------------------ host helpers --
def _support_radius(cpoint_loc):
    diff = cpoint_loc[:, :, None, :] - cpoint_loc[:, None, :, :]
    sq = (diff * diff).sum(-1)
    sq = sq + np.eye(N, dtype=sq.dtype) * BIG
    return np.sqrt(sq.min(2).max(1)) * C_FACTOR  # [B]


def _split3(v):
    h = v.astype(np.float16)
    r = v - h.astype(np.float32)
    m = r.astype(np.float16)
    l = (r - m.astype(np.float32)).astype(np.float16)
    return h, m, l


def _point_features(px, py):
    """[12, K] fp16: [4,4,4, s4h,s4m,s4l, -2pxh,-2pxm,-2pxl, -2pyh,-2pym,-2pyl]"""
    sp4 = (px * px + py * py + EPS) * np.float32(0.25)
    s4h, s4m, s4l = _split3(sp4)
    four = np.full_like(px, 4.0, dtype=np.float16)

    def m2(v):
        return (np.float32(-2.0) * v.astype(np.float32)).astype(np.float16)

    pxh, pxm, pxl = _split3(px)
    pyh, pym, pyl = _split3(py)
    return np.stack([four, four, four, s4h, s4m, s4l,
                     m2(pxh), m2(pxm), m2(pxl),
                     m2(pyh), m2(pym), m2(pyl)]).astype(np.float16)


def _pixel_features():
    """[NBANDS, 12, R_BAND*W] fp16 pixel features, row-major per band."""
    yy, xx = np.meshgrid(np.arange(H, dtype=np.float32),
                         np.arange(W, dtype=np.float32), indexing='ij')
    x = xx.reshape(-1)
    y = yy.reshape(-1)
    gp4 = (x * x + y * y) * np.float32(0.25)
    g4h, g4m, g4l = _split3(gp4)
    four = np.full_like(x, 4.0, dtype=np.float16)
    xb = x.astype(np.float16)
    yb = y.astype(np.float16)
    feats = np.stack([g4h, g4m, g4l, four, four, four,
                      xb, xb, xb, yb, yb, yb]).astype(np.float16)  # [12, H*W]
    return feats.reshape(12, NBANDS, R_BAND * W).transpose(1, 0, 2).copy()


def _gather_bands(cp, al, c, kpts):
    """Per band: active point features [12,kpts] fp16 and 5a/-4a [kpts,2] f32.
    Returns (ptf [NBANDS,12,kpts], a5 [NBANDS,kpts,2], a4 [NBANDS,kpts,2],
    max_active)."""
    px, py = cp[:, 0], cp[:, 1]
    ptf = np.zeros((NBANDS, 12, kpts), np.float16)
    a5 = np.zeros((NBANDS, kpts, 2), np.float32)
    a4 = np.zeros((NBANDS, kpts, 2), np.float32)
    max_active = 0
    for i in range(NBANDS):
        y0, y1 = i * R_BAND, i * R_BAND + R_BAND - 1
        dy = np.maximum(np.maximum(y0 - py, py - y1), 0.0)
        idx = np.nonzero(dy < c)[0]
        max_active = max(max_active, len(idx))
        if len(idx) > kpts:
            return None, None, None, max_active
        gx = np.full(kpts, PAD_XY, np.float32)
        gy = np.full(kpts, PAD_XY, np.float32)
        gx[:len(idx)] = px[idx]
        gy[:len(idx)] = py[idx]
        ptf[i] = _point_features(gx, gy)
        a5[i, :len(idx)] = al[idx]
        a4[i, :len(idx)] = al[idx]
    return ptf, a5, a4, max_active


# ------------------------------------------------------------ build module --
def _build_module_flat(kpts):
    """One 4-row band (1024 px) per column group; kpts<=128 gathered points
    on partitions. DMAs batched: all weights preloaded, pixel features in
    8-band chunks, outputs staged in SBUF and written per 16 bands.
    Contraction accumulates 4 bands into one [8, FG] psum via zero-padded
    block lhsT; ScalarE: sqrt + relu; VectorE: q/t4/t5; GpSimd: psum->stage
    copies."""
    import concourse.bass as bass
    import concourse.tile as tile
    import concourse.mybir as mybir

    NGRP = NBANDS // 4           # 16 groups of 4 bands
    NSTG = 4                     # groups per output stage buffer

    nc = bass.Bass()
    f16, f32 = mybir.dt.float16, mybir.dt.float32
    pxf_d = nc.dram_tensor("pxf", [NBANDS, 12, FG], f16, kind="ExternalInput")
    ptf_d = nc.dram_tensor("ptf", [NBANDS, 12, kpts], f16, kind="ExternalInput")
    al5_d = nc.dram_tensor("al5", [NBANDS, kpts, 8], f16, kind="ExternalInput")
    al4_d = nc.dram_tensor("al4", [NBANDS, kpts, 8], f16, kind="ExternalInput")
    sc_d = nc.dram_tensor("sc", [128, 1], f32, kind="ExternalInput")
    out_d = nc.dram_tensor("out", [CH, H * W], f32, kind="ExternalOutput")

    with tile.TileContext(nc) as tc:
        with (
            tc.tile_pool(name="weights", bufs=1) as wpool,
            tc.tile_pool(name="io", bufs=2) as iopool,
            tc.tile_pool(name="chain", bufs=3) as chpool,
            tc.tile_pool(name="stage", bufs=2) as stpool,
            tc.tile_pool(name="psum", bufs=2, space=bass.MemorySpace.PSUM) as pspool,
        ):
            s_sc = wpool.tile([128, 1], f32, tag="sc")
            nc.sync.dma_start(s_sc[:], sc_d[:])
            s_ptf = wpool.tile([12, NBANDS * kpts], f16, tag="ptf")
            nc.sync.dma_start(
                s_ptf.rearrange("f (b k) -> f b k", b=NBANDS),
                ptf_d.rearrange("b f k -> f b k"))
            s_al5 = wpool.tile([kpts, NBANDS * 8], f16, tag="al5")
            nc.sync.dma_start(
                s_al5.rearrange("k (b c) -> k b c", b=NBANDS),
                al5_d.rearrange("b k c -> k b c"))
            s_al4 = wpool.tile([kpts, NBANDS * 8], f16, tag="al4")
            nc.sync.dma_start(
                s_al4.rearrange("k (b c) -> k b c", b=NBANDS),
                al4_d.rearrange("b k c -> k b c"))

            for G in range(NGRP):
                if G % 2 == 0:
                    # pixel features for the next 8 bands in one DMA
                    s_pxf = iopool.tile([12, 8 * FG], f16, tag="pxf")
                    nc.sync.dma_start(
                        s_pxf.rearrange("f (b p) -> f b p", b=8),
                        pxf_d[8 * (G // 2):8 * (G // 2) + 8].rearrange(
                            "b f p -> f b p"))
                if G % NSTG == 0:
                    stage = stpool.tile([8, NSTG * FG], f32, tag="stage")

                ps_out = pspool.tile([8, FG], f32, tag="out")
                for half in range(2):                 # two band-pairs per group
                    i0 = 2 * half
                    bandP = [4 * G + i0, 4 * G + i0 + 1]
                    # wide chain tiles covering both bands (2*FG columns)
                    d_t = chpool.tile([kpts, 2 * FG], f16, tag="d")
                    t_t = chpool.tile([kpts, 2 * FG], f16, tag="t")
                    q = chpool.tile([kpts, 2 * FG], f16, tag="q")
                    t4 = chpool.tile([kpts, 2 * FG], f16, tag="t4")
                    t5 = chpool.tile([kpts, 2 * FG], f16, tag="t5")
                    for k, band in enumerate(bandP):
                        ptcol = slice(band * kpts, band * kpts + kpts)
                        ps_d2 = pspool.tile([kpts, FG], f32, tag="d2")
                        for j in range(FG // 512):
                            sl = slice(j * 512, (j + 1) * 512)
                            pxsl = slice((band % 8) * FG + j * 512,
                                         (band % 8) * FG + (j + 1) * 512)
                            nc.tensor.matmul(ps_d2[:, sl], s_ptf[:, ptcol],
                                             s_pxf[:, pxsl], start=True,
                                             stop=True)
                        nc.scalar.activation(
                            d_t[:, k * FG:(k + 1) * FG], ps_d2[:],
                            mybir.ActivationFunctionType.Sqrt,
                            bias=0.0, scale=s_sc[0:kpts, :])
                    nc.scalar.activation(t_t[:], d_t[:],
                                         mybir.ActivationFunctionType.Relu,
                                         bias=1.0, scale=-1.0)
                    nc.vector.tensor_tensor(q[:], t_t[:], t_t[:],
                                            mybir.AluOpType.mult)
                    nc.vector.tensor_tensor(t4[:], q[:], q[:],
                                            mybir.AluOpType.mult)
                    nc.vector.tensor_tensor(t5[:], t4[:], t_t[:],
                                            mybir.AluOpType.mult)
                    for k, band in enumerate(bandP):
                        i = i0 + k
                        alcol = slice(band * 8, band * 8 + 8)
                        for j in range(FG // 512):
                            sl = slice(j * 512, (j + 1) * 512)
                            wsl = slice(k * FG + j * 512, k * FG + (j + 1) * 512)
                            nc.tensor.matmul(ps_out[:, sl], s_al5[:, alcol],
                                             t4[:, wsl], start=(i == 0),
                                             stop=False)
                        for j in range(FG // 512):
                            sl = slice(j * 512, (j + 1) * 512)
                            wsl = slice(k * FG + j * 512, k * FG + (j + 1) * 512)
                            nc.tensor.matmul(ps_out[:, sl], s_al4[:, alcol],
                                             t5[:, wsl], start=False,
                                             stop=(i == 3))

                gg = G % NSTG
                nc.vector.tensor_copy(stage[:, gg * FG:(gg + 1) * FG], ps_out[:])
                if gg == NSTG - 1:
                    # stage[2i+ch, gg*FG+p] -> out[ch, base + gg*4FG + i*FG + p]
                    base = (G - NSTG + 1) * 4 * FG
                    st_r = stage.rearrange("(i r) (g p) -> r i g p", r=2, g=NSTG)
                    for ch in range(CH):
                        dst = out_d[ch, base:base + 4 * NSTG * FG].rearrange(
                            "(g i p) -> i g p", g=NSTG, i=4, p=FG)
                        nc.sync.dma_start(dst, st_r[ch])
    return nc


# ------------------------------------------------------------------ kernel --
def kernel(cpoint_loc: np.ndarray, alpha: np.ndarray) -> np.ndarray:
    assert cpoint_loc.shape == (B, N, 2) and alpha.shape == (B, N, CH)
    cp = np.asarray(cpoint_loc, np.float32)
    al = np.asarray(alpha, np.float32)

    _apply_patches()
    from concourse.bass_utils import run_bass_kernel_spmd

    c = _support_radius(cp)                      # [B]
    pxf = _pixel_features()                      # shared [NBANDS, 12, FG]

    # probe active counts, pick a uniform kpts across cores
    mx_all = 0
    for b in range(B):
        _, _, _, mx = _gather_bands(cp[b], al[b], c[b], 160)
        mx_all = max(mx_all, mx)
    if mx_all > 128:
        raise NotImplementedError(
            f"band active count {mx_all} > 128 unsupported")
    kpts = max(32, int(np.ceil(mx_all / 32.0)) * 32)
    gathered = [_gather_bands(cp[b], al[b], c[b], kpts) for b in range(B)]

    key = ("flat", kpts)
    if key not in _CACHE:
        _CACHE[key] = _build_module_flat(kpts)
    nc = _CACHE[key]

    in_maps = []
    for b in range(B):
        ptf, a5, a4, _ = gathered[b]
        sc = np.full((128, 1), 1.0 / (c[b] * c[b]), np.float32)
        # block layout: band i of its 4-band group owns lhsT cols 2i..2i+1
        al5 = np.zeros((NBANDS, kpts, 8), np.float16)
        al4 = np.zeros((NBANDS, kpts, 8), np.float16)
        for band in range(NBANDS):
            i = band % 4
            al5[band, :, 2 * i:2 * i + 2] = (5.0 * a5[band]).astype(np.float16)
            al4[band, :, 2 * i:2 * i + 2] = (-4.0 * a4[band]).astype(np.float16)
        in_maps.append({"pxf": pxf, "ptf": ptf, "al5": al5, "al4": al4,
                        "sc": sc})

    trace = bool(os.environ.get("RBF_TRACE")) and _maybe_install_trace_hook()
    res = run_bass_kernel_spmd(nc, in_maps, core_ids=list(range(B)), trace=trace)
    global LAST_EXEC_TIME_NS
    if res.exec_time_ns is not None:
        LAST_EXEC_TIME_NS = res.exec_time_ns
    out = np.stack([r["out"] for r in res.results])     # [B, 2, H*W]
    return out.reshape(B, CH, H, W).astype(np.float32)


if __name__ == "__main__":
    rng = np.random.default_rng(0)
    cp = rng.random((B, N, 2), np.float32) * np.array([W - 1, H - 1], np.float32)
    a = rng.standard_normal((B, N, CH)).astype(np.float32)
    out = kernel(cp, a)
    print(out.shape, np.abs(out).max())
